# revision 5
# baseline (speedup 1.0000x reference)
"""DisenEncoder (disentangled GNN routing) Trainium2 kernel.

Strategy: shard nodes (and therefore edges, by *target*) across 8 cores.
Since z = h[src] is fixed before the routing loop and each edge only
updates / reads the state c of its *target* node, partitioning edges by
target makes all 6 routing iterations fully core-local (no collectives).

Per core:
  init:  h = l2norm((x @ W.T + b).reshape(n,4,32))   (replicated, full n)
         z  = h[src]  materialized lane-major in DRAM
         c0 = h[shard]
  loop (x6), per 128-target tile:
         ctrg   = c[trg]               (indirect DMA gather, ping-pong DRAM)
         p      = softmax_k(sum_dd z*ctrg)
         ws     = p * z
         delta  = onehot_et.T @ ws     (PE segment-sum, PSUM accumulate)
         c_tile = l2norm(c_tile + delta)
"""

import sys
from types import SimpleNamespace

import numpy as np

if "/opt/trn_rl_repo" not in sys.path:
    sys.path.insert(0, "/opt/trn_rl_repo")

import concourse.bass as bass
from concourse import bacc, mybir, tile
from concourse.bass import ds, ts
from concourse.bass_utils import run_bass_kernel_spmd

AF = mybir.ActivationFunctionType
ALU = mybir.AluOpType
F32 = mybir.dt.float32
I32 = mybir.dt.int32

EPS2 = 1e-24  # added under the sqrt; matches reference's max(norm, 1e-12)


def full_params():
    P = SimpleNamespace()
    P.N = 100000      # nodes
    P.D = 128         # feature dim (= K * DD)
    P.K = 4
    P.DD = 32
    P.NCORES = 8
    P.SH = P.N // P.NCORES          # nodes per core (12500)
    P.NT = (P.SH + 127) // 128      # target tiles per core (98)
    P.CT = P.NT * 128               # padded c rows per core (12544)
    P.J = 18                        # edge chunks (x128) per target tile
    P.JT = P.NT * P.J
    P.F = P.J * 128                 # edge slots per tile (2304)
    P.ROUTIT = 6
    P.HB = 16                       # h-loop unroll (node tiles per body)
    ntx = (P.N + 1 + 127) // 128    # +1: need >=1 zero pad row
    P.NTX = ((ntx + P.HB - 1) // P.HB) * P.HB
    P.NH = P.NTX * 128              # padded h rows (100352)
    return P


def _bc_last(ap, n):
    """[P, ...G] view -> [P, ...G, n] with stride-0 broadcast last dim."""
    return bass.AP(ap.tensor, ap.offset, [list(e) for e in ap.ap] + [[0, n]])


def _bc_mid(ap, n):
    """[P, T] view -> [P, n, T] with stride-0 broadcast middle dim."""
    aps = [list(e) for e in ap.ap]
    return bass.AP(ap.tensor, ap.offset, [aps[0], [0, n]] + aps[1:])


def build_program(P):
    nc = bacc.Bacc(None, target_bir_lowering=False)

    xT = nc.dram_tensor("xT", [128, P.NH], F32, kind="ExternalInput")
    wT = nc.dram_tensor("wT", [128, 128], F32, kind="ExternalInput")
    bias = nc.dram_tensor("bias", [1, 128], F32, kind="ExternalInput")
    srcT = nc.dram_tensor("srcT", [128, P.JT], I32, kind="ExternalInput")
    tabsT = nc.dram_tensor("tabsT", [128, P.JT], I32, kind="ExternalInput")
    trelT = nc.dram_tensor("trelT", [128, P.JT], F32, kind="ExternalInput")
    h = nc.dram_tensor("h", [P.NH, 128], F32)
    zlane = nc.dram_tensor("zlane", [128, P.NT * P.F], F32)
    c_out = nc.dram_tensor("c_out", [P.CT, 128], F32, kind="ExternalOutput")
    c_alt = nc.dram_tensor("c_alt", [P.CT, 128], F32)

    with tile.TileContext(nc) as tc:
        with (
            tc.tile_pool(name="const", bufs=1) as const,
            tc.tile_pool(name="big", bufs=2) as big,
            tc.tile_pool(name="small", bufs=3) as small,
            tc.tile_pool(name="psum", bufs=2, space="PSUM") as psum,
        ):
            w_sb = const.tile([128, 128], F32)
            nc.sync.dma_start(w_sb[:], wT[:, :])
            b_sb = const.tile([1, 128], F32)
            nc.sync.dma_start(b_sb[:], bias[:, :])
            ones1 = const.tile([1, 128], F32)
            nc.vector.memset(ones1[:], 1.0)
            iota = const.tile([128, 128], F32)
            nc.gpsimd.iota(
                iota[:], [[1, 128]], channel_multiplier=0,
                allow_small_or_imprecise_dtypes=True,
            )
            zeros = const.tile([128, 128], F32)
            nc.vector.memset(zeros[:], 0.0)
            eps_sb = const.tile([128, 1], F32)
            nc.vector.memset(eps_sb[:], EPS2)

            # ---- phase 0a: h = l2norm(x @ W.T + b) (full graph, replicated)
            def h_tile(texpr):
                xt = small.tile([128, 128], F32, tag="xt")
                nc.sync.dma_start(xt[:], xT[:, ds(texpr * 128, 128)])
                hp = psum.tile([128, 128], F32, tag="hp")
                nc.tensor.matmul(hp[:], lhsT=xt[:], rhs=w_sb[:], start=True, stop=False)
                nc.tensor.matmul(hp[:], lhsT=ones1[:], rhs=b_sb[:], start=False, stop=True)
                _normalize_to(nc, small, hp, h, ds(texpr * 128, 128))

            def _normalize_to(nc, pool, src_tile, dram, row_slice):
                sq = pool.tile([128, 128], F32, tag="sq")
                nc.scalar.activation(sq[:], src_tile[:], AF.Square)
                s4 = pool.tile([128, 4], F32, tag="s4")
                nc.vector.tensor_reduce(
                    s4[:], sq[:].rearrange("p (k d) -> p k d", d=32),
                    axis=mybir.AxisListType.X, op=ALU.add,
                )
                nr = pool.tile([128, 4], F32, tag="nr")
                nc.scalar.activation(nr[:], s4[:], AF.Sqrt, bias=eps_sb[:])
                rv = pool.tile([128, 4], F32, tag="rv")
                nc.vector.reciprocal(rv[:], nr[:])
                out_sb = pool.tile([128, 128], F32, tag="outn")
                nc.vector.tensor_tensor(
                    out_sb[:].rearrange("p (k d) -> p k d", d=32),
                    src_tile[:].rearrange("p (k d) -> p k d", d=32),
                    _bc_last(rv[:], 32),
                    op=ALU.mult,
                )
                nc.sync.dma_start(dram[row_slice, :], out_sb[:])

            with tc.For_i(0, P.NTX, P.HB) as tb:
                for u in range(P.HB):
                    h_tile(tb + u)

            # zero the h pad rows (so padded-edge z gathers read zeros)
            r = P.N
            while r < P.NH:
                c = min(128, P.NH - r)
                nc.sync.dma_start(h[r:r + c, :], zeros[:c, :])
                r += c

            # ---- phase 0b: materialize z = h[src], lane-major
            with tc.For_i(0, P.NT) as t:
                sidx = small.tile([128, P.J], I32, tag="sidx")
                nc.sync.dma_start(sidx[:], srcT[:, ds(t * P.J, P.J)])
                zt = big.tile([128, P.F], F32, tag="zt")
                for j in range(P.J):
                    nc.gpsimd.indirect_dma_start(
                        out=zt[:, j * 128:(j + 1) * 128],
                        out_offset=None,
                        in_=h[:, :],
                        in_offset=bass.IndirectOffsetOnAxis(ap=sidx[:, j:j + 1], axis=0),
                    )
                nc.sync.dma_start(zlane[:, ds(t * P.F, P.F)], zt[:])

            # ---- phase 0c: c0 = h[shard]  (per-core offset via partition id)
            pid = nc.sync.partition_id()
            for t in range(P.NT):
                cc = small.tile([128, 128], F32, tag="cinit")
                nc.sync.dma_start(cc[:], h[ds(pid * P.SH + t * 128, 128), :])
                nc.sync.dma_start(c_out[ts(t, 128), :], cc[:])

            # ---- phase 1: routing iterations
            for it in range(P.ROUTIT):
                cread, cwrite = (c_out, c_alt) if it % 2 == 0 else (c_alt, c_out)
                with tc.For_i(0, P.NT) as t:
                    tabs = small.tile([128, P.J], I32, tag="tabs")
                    nc.sync.dma_start(tabs[:], tabsT[:, ds(t * P.J, P.J)])
                    trel = small.tile([128, P.J], F32, tag="trel")
                    nc.sync.dma_start(trel[:], trelT[:, ds(t * P.J, P.J)])
                    zt = big.tile([128, P.F], F32, tag="zt")
                    nc.sync.dma_start(zt[:], zlane[:, ds(t * P.F, P.F)])
                    ct = big.tile([128, P.F], F32, tag="ct")
                    for j in range(P.J):
                        nc.gpsimd.indirect_dma_start(
                            out=ct[:, j * 128:(j + 1) * 128],
                            out_offset=None,
                            in_=cread[:, :],
                            in_offset=bass.IndirectOffsetOnAxis(
                                ap=tabs[:, j:j + 1], axis=0),
                        )
                    prod = big.tile([128, P.F], F32, tag="prod")
                    nc.vector.tensor_tensor(prod[:], zt[:], ct[:], op=ALU.mult)
                    praw = small.tile([128, P.J * 4], F32, tag="praw")
                    nc.vector.tensor_reduce(
                        praw[:], prod[:].rearrange("p (g d) -> p g d", d=32),
                        axis=mybir.AxisListType.X, op=ALU.add,
                    )
                    pexp = small.tile([128, P.J * 4], F32, tag="pexp")
                    nc.scalar.activation(pexp[:], praw[:], AF.Exp)
                    psk = small.tile([128, P.J], F32, tag="psk")
                    nc.vector.tensor_reduce(
                        psk[:], pexp[:].rearrange("p (j k) -> p j k", k=4),
                        axis=mybir.AxisListType.X, op=ALU.add,
                    )
                    piv = small.tile([128, P.J], F32, tag="piv")
                    nc.vector.reciprocal(piv[:], psk[:])
                    pn = small.tile([128, P.J * 4], F32, tag="pn")
                    nc.vector.tensor_tensor(
                        pn[:].rearrange("p (j k) -> p j k", k=4),
                        pexp[:].rearrange("p (j k) -> p j k", k=4),
                        _bc_last(piv[:], 4),
                        op=ALU.mult,
                    )
                    ws = big.tile([128, P.F], F32, tag="ws")
                    nc.vector.tensor_tensor(
                        ws[:].rearrange("p (g d) -> p g d", d=32),
                        zt[:].rearrange("p (g d) -> p g d", d=32),
                        _bc_last(pn[:], 32),
                        op=ALU.mult,
                    )
                    oh = big.tile([128, P.F], F32, tag="oh")
                    nc.vector.tensor_tensor(
                        oh[:].rearrange("p (j t) -> p j t", t=128),
                        _bc_last(trel[:], 128),
                        _bc_mid(iota[:], P.J),
                        op=ALU.is_equal,
                    )
                    dps = psum.tile([128, 128], F32, tag="dps")
                    for j in range(P.J):
                        nc.tensor.matmul(
                            dps[:],
                            lhsT=oh[:, j * 128:(j + 1) * 128],
                            rhs=ws[:, j * 128:(j + 1) * 128],
                            start=(j == 0), stop=(j == P.J - 1),
                        )
                    cold = small.tile([128, 128], F32, tag="cold")
                    nc.sync.dma_start(cold[:], cread[ds(t * 128, 128), :])
                    cd = small.tile([128, 128], F32, tag="cd")
                    nc.vector.tensor_tensor(cd[:], cold[:], dps[:], op=ALU.add)
                    _normalize_to(nc, small, cd, cwrite, ds(t * 128, 128))

    nc.finalize()
    return nc


def prep_host(P, x, src_trg, W, b):
    """Build per-core padded, tile-sorted edge arrays + shared tensors."""
    src = np.asarray(src_trg[0]).astype(np.int64)
    trg = np.asarray(src_trg[1]).astype(np.int64)
    x = np.asarray(x, dtype=np.float32)
    W = np.asarray(W, dtype=np.float32)
    b = np.asarray(b, dtype=np.float32)

    xT = np.zeros((128, P.NH), dtype=np.float32)
    xT[:, :P.N] = x.T
    wT = np.ascontiguousarray(W.T)          # [in_dim, d]
    bias = b.reshape(1, 128).astype(np.float32)

    shared = {"xT": xT, "wT": wT, "bias": bias}

    in_maps = []
    for core in range(P.NCORES):
        lo, hi = core * P.SH, (core + 1) * P.SH
        sel = (trg >= lo) & (trg < hi)
        s = src[sel]
        t = trg[sel] - lo
        tile_id = t >> 7
        order = np.argsort(tile_id, kind="stable")
        s, t, tile_id = s[order], t[order], tile_id[order]
        counts = np.bincount(tile_id, minlength=P.NT)
        assert counts.max() <= P.J * 128, (
            f"tile overflow: {counts.max()} > {P.J * 128}")

        S = P.JT * 128
        src_pad = np.full(S, P.N, dtype=np.int32)       # pad -> zero h row
        tabs_pad = np.full(S, min(P.SH, P.CT - 1), dtype=np.int32)
        trel_pad = np.full(S, -1.0, dtype=np.float32)   # never matches iota
        ofs = 0
        for tt in range(P.NT):
            cnt = int(counts[tt])
            base = tt * P.J * 128
            src_pad[base:base + cnt] = s[ofs:ofs + cnt]
            tabs_pad[base:base + cnt] = t[ofs:ofs + cnt]
            trel_pad[base:base + cnt] = (t[ofs:ofs + cnt] - tt * 128).astype(
                np.float32)
            ofs += cnt

        m = dict(shared)
        m["srcT"] = np.ascontiguousarray(src_pad.reshape(P.JT, 128).T)
        m["tabsT"] = np.ascontiguousarray(tabs_pad.reshape(P.JT, 128).T)
        m["trelT"] = np.ascontiguousarray(trel_pad.reshape(P.JT, 128).T)
        in_maps.append(m)
    return in_maps


def kernel_traced(trace=False, **inputs):
    P = full_params()
    x, src_trg, W, b = inputs["x"], inputs["src_trg"], inputs["W"], inputs["b"]
    in_maps = prep_host(P, x, src_trg, W, b)
    nc = build_program(P)
    res = run_bass_kernel_spmd(
        nc, in_maps, core_ids=list(range(P.NCORES)), trace=trace)
    out = np.empty((P.N, 128), dtype=np.float32)
    for core in range(P.NCORES):
        out[core * P.SH:(core + 1) * P.SH] = res.results[core]["c_out"][:P.SH]
    return out, res


def kernel(**inputs):
    out, _ = kernel_traced(trace=False, **inputs)
    return out
